# revision 45
# baseline (speedup 1.0000x reference)
"""AttnBlock (GroupNorm -> single-head spatial attention -> out-proj -> residual)
as a Trainium2 Bass/Tile kernel, SPMD over 8 NeuronCores.

Sharding: 4 samples x 2 q-halves = 8 shards. Each core receives one sample's
[C, N] activation map, column-rotated so that the core's q-half is always
columns 0..NQ-1 (attention is permutation-invariant over k and GroupNorm
stats are permutation-invariant, so rotation is free).

Algebraic folds (all exact up to fp rounding):
  - bk and the k-side GN-bias term drop out of softmax (per-q shift
    invariance). No max subtraction: logits are O(5), fp32 exp is safe,
    softmax is shift-invariant so this matches the reference.
  - The GN channel affine h = sc*x + bi is never materialized:
      * q/k projection QK2[ci,q] = sc_ci * ((WM*sc)@x_q + bM + WM@bi),
        folded into weight staging + the PSUM->SBUF ACT copy.
      * scores are computed off raw (rounded) x: S^T = x^T QK2, in [k,q]
        layout so the softmax denominator is a ones-vector matmul.
      * value/output path: out = ((WF*sc)@(x@A^T)) * r + (WF@bi + bF) + x,
        because sum_k A_norm = 1 pushes bi through attention, and the
        per-q normalizer r commutes through the channel-mixing projection.
  - WMT = wq.T @ wk, WFT = (wo @ wv).T, bM = wk.T @ bq, bF = wo @ bv + bo:
    host-side weight preprocessing.

Matmul dtype: float32r (4x faster PE path; every matmul operand is produced
by a compute engine writing fp32r, satisfying the BIR verifier's rounding
rule). Set ATTN_MM_DT=float32 for the exact (4x slower) variant.
"""

import os

import numpy as np

import concourse.bacc as bacc
import concourse.mybir as mybir
from concourse.tile import TileContext
from concourse.bass_utils import run_bass_kernel_spmd

P = 128
C = 512
N = 4096          # h*w spatial positions per sample
NQ = 2048         # q positions per core (half a sample)
NCH = C // P      # 4 channel chunks
NK = N // P       # 32 k chunks
NQC = NQ // 512   # 4 q chunks of 512
GROUP = 16        # channels per group (512 / 32 groups)
EPS = 1e-6
SM_SCALE = 1.0 / float(np.sqrt(C))

F32 = mybir.dt.float32
MDT = (mybir.dt.float32 if os.environ.get("ATTN_MM_DT") == "float32"
       else mybir.dt.float32r)

_CACHE = {}


def build_module():
    """Build (and cache) the compiled Bass module for one core."""
    if "nc" in _CACHE:
        return _CACHE["nc"]

    nc = bacc.Bacc("TRN2", target_bir_lowering=False, debug=False)
    Id = mybir.ActivationFunctionType.Identity
    Exp = mybir.ActivationFunctionType.Exp
    Sqrt = mybir.ActivationFunctionType.Sqrt
    Add = mybir.AluOpType.add
    mm = nc.tensor.matmul

    xf = nc.dram_tensor("xf", [C, N], F32, kind="ExternalInput").ap()
    wmt_d = nc.dram_tensor("wmt", [C, C], F32, kind="ExternalInput").ap()
    wft_d = nc.dram_tensor("wft", [C, C], F32, kind="ExternalInput").ap()
    # columns: [bm, bf, gamma, beta]
    biasc_d = nc.dram_tensor("biasc", [C, 4], F32, kind="ExternalInput").ap()
    gmat_d = nc.dram_tensor("gmat", [P, P], F32, kind="ExternalInput").ap()
    idt_d = nc.dram_tensor("idt", [P, P], F32, kind="ExternalInput").ap()
    out_d = nc.dram_tensor("out", [C, NQ], F32, kind="ExternalOutput").ap()

    with TileContext(nc) as tc:
        with (
            tc.tile_pool(name="consts", bufs=1) as cpool,
            tc.tile_pool(name="big", bufs=1) as big,
            tc.tile_pool(name="gnw", bufs=2) as gnw,
            tc.tile_pool(name="mmps", bufs=3, space="PSUM") as mmps,
            tc.tile_pool(name="zps", bufs=1, space="PSUM") as zps,
            tc.tile_pool(name="sps", bufs=1, space="PSUM") as sps,
        ):
            # ---- small constants (x quarters get the sync queue head) ----
            gmat = cpool.tile([P, P], F32, tag="gmat")
            ones_k = cpool.tile([P, 1], MDT, tag="ones_k")
            ones_m = cpool.tile([1, P], MDT, tag="ones_m")
            eps_t = cpool.tile([P, 1], F32, tag="eps")
            nc.vector.memset(eps_t, EPS)

            bm_t, bf_t, gam_t, bet_t = [], [], [], []
            bc_tiles = []
            for j in range(NCH):
                bc = cpool.tile([P, 4], F32, tag=f"bc{j}", name=f"bc{j}")
                bc_tiles.append(bc)
                bm_t.append(bc[:, 0:1])
                bf_t.append(bc[:, 1:2])
                gam_t.append(bc[:, 2:3])
                bet_t.append(bc[:, 3:4])

            wmq_pool = tc.tile_pool(name="wmq", bufs=1)
            wmq = wmq_pool.__enter__()
            wmt2 = [wmq.tile([P, C], MDT, tag=f"wmt{j}", name=f"wmt{j}")
                    for j in range(NCH)]
            wft2 = [cpool.tile([P, C], MDT, tag=f"wft{j}", name=f"wft{j}")
                    for j in range(NCH)]
            idt = cpool.tile([P, P], MDT, tag="idtm")
            sc_t = [cpool.tile([P, 1], F32, tag=f"sc{j}", name=f"sc{j}")
                    for j in range(NCH)]
            bi_t = [cpool.tile([P, 1], F32, tag=f"bi{j}", name=f"bi{j}")
                    for j in range(NCH)]
            b2_t = [cpool.tile([P, 1], F32, tag=f"b2{j}", name=f"b2{j}")
                    for j in range(NCH)]
            bff_t = [cpool.tile([P, 1], F32, tag=f"bff{j}", name=f"bff{j}")
                     for j in range(NCH)]

            xm = [big.tile([P, N], MDT, tag=f"xm{j}", name=f"xm{j}")
                  for j in range(NCH)]

            with tc.tile_pool(name="stage", bufs=1) as stage:
                # identity first: it gates the first PE transpose
                wsi = stage.tile([P, P], F32, tag="wsi", name="wsi")
                nc.sync.dma_start(out=wsi, in_=idt_d)
                nc.scalar.copy(out=idt, in_=wsi)
                ones_f = stage.tile([P, 1], F32, tag="ones_f", name="ones_f")
                nc.vector.memset(ones_f, 1.0)
                nc.scalar.copy(out=ones_k, in_=ones_f)
                ones_mf = stage.tile([1, P], F32, tag="ones_mf", name="ones_mf")
                nc.vector.memset(ones_mf, 1.0)
                nc.scalar.copy(out=ones_m, in_=ones_mf)

                # x: quarter-major load; bn_stats (fp32) + rounded copy to xm
                stats = [gnw.tile([P, 8, 6], F32, tag=f"stats{j}",
                                  name=f"stats{j}", bufs=1)
                         for j in range(NCH)]
                for t in range(8):
                    cs = slice(t * 512, (t + 1) * 512)
                    for j in range(NCH):
                        xq = stage.tile([P, 512], F32, tag="xq", name="xq",
                                        bufs=5)
                        eng = (nc.sync if t == 0 or (t * NCH + j) % 2 == 0
                               else nc.gpsimd)
                        eng.dma_start(out=xq, in_=xf[j * P:(j + 1) * P, cs])
                        nc.vector.bn_stats(out=stats[j][:, t, :], in_=xq)
                        nc.scalar.copy(out=xm[j][:, cs], in_=xq)
                    if t == 0:
                        nc.gpsimd.dma_start(out=gmat, in_=gmat_d)
                        for j in range(NCH):
                            nc.gpsimd.dma_start(
                                out=bc_tiles[j],
                                in_=biasc_d[j * P:(j + 1) * P, :])

                # raw weights (whole; scaled/copied once sc is known) --
                # loaded after x so they don't delay the first transposes
                wsm = [stage.tile([P, C], F32, tag=f"wm{j}", name=f"wm{j}")
                       for j in range(NCH)]
                wsf = [stage.tile([P, C], F32, tag=f"wf{j}", name=f"wf{j}")
                       for j in range(NCH)]
                for j in range(NCH):
                    r = slice(j * P, (j + 1) * P)
                    nc.sync.dma_start(out=wsm[j], in_=wmt_d[r, :])
                    nc.sync.dma_start(out=wsf[j], in_=wft_d[r, :])

                # channel stats -> group stats -> per-channel sc/bi
                for j in range(NCH):
                    mv = gnw.tile([P, 2], F32, tag="mv", name="mv")
                    nc.vector.bn_aggr(out=mv, in_=stats[j])
                    mv2 = gnw.tile([P, 2], F32, tag="mv2", name="mv2")
                    nc.vector.tensor_copy(out=mv2[:, 0:1], in_=mv[:, 0:1])
                    nc.vector.tensor_mul(out=mv2[:, 1:2], in0=mv[:, 0:1],
                                         in1=mv[:, 0:1])
                    nc.vector.tensor_add(out=mv2[:, 1:2], in0=mv2[:, 1:2],
                                         in1=mv[:, 1:2])
                    gs = sps.tile([P, 2], F32, tag="sums", name="gs")
                    mm(gs, gmat, mv2, start=True, stop=True)
                    gmean = gnw.tile([P, 1], F32, tag="gmean", name="gmean")
                    nc.vector.tensor_scalar_mul(gmean, gs[:, 0:1], 1.0 / GROUP)
                    gvar = gnw.tile([P, 1], F32, tag="gvar", name="gvar")
                    nc.vector.tensor_scalar_mul(gvar, gs[:, 1:2], 1.0 / GROUP)
                    tmp = gnw.tile([P, 1], F32, tag="tmp", name="tmp")
                    nc.vector.tensor_mul(out=tmp, in0=gmean, in1=gmean)
                    nc.vector.tensor_sub(out=gvar, in0=gvar, in1=tmp)
                    std = gnw.tile([P, 1], F32, tag="std", name="std")
                    nc.scalar.activation(out=std, in_=gvar, func=Sqrt, bias=eps_t)
                    rstd = gnw.tile([P, 1], F32, tag="rstd", name="rstd")
                    nc.vector.reciprocal(out=rstd, in_=std)
                    nc.vector.tensor_mul(out=sc_t[j], in0=rstd, in1=gam_t[j])
                    nc.vector.tensor_mul(out=bi_t[j], in0=gmean, in1=sc_t[j])
                    nc.vector.tensor_sub(out=bi_t[j], in0=bet_t[j], in1=bi_t[j])

                # scaled weight copies + device-side bias folds
                for j in range(NCH):
                    nc.vector.tensor_scalar_mul(wmt2[j], wsm[j], sc_t[j])
                    nc.scalar.mul(out=wft2[j], in_=wsf[j], mul=sc_t[j])
                for ci in range(NCH):
                    # b2 = sc * (bM + WM @ bi);  bff = WF @ bi + bF
                    b2p = sps.tile([P, 1], F32, tag="sums", name="b2p")
                    for j in range(NCH):
                        mm(b2p, wsm[j][:, ci * P:(ci + 1) * P], bi_t[j],
                           start=(j == 0), stop=(j == NCH - 1))
                    nc.vector.tensor_add(out=b2_t[ci], in0=b2p, in1=bm_t[ci])
                    nc.vector.tensor_mul(out=b2_t[ci], in0=b2_t[ci],
                                         in1=sc_t[ci])
                    bfp = sps.tile([P, 1], F32, tag="sums", name="bfp")
                    for j in range(NCH):
                        mm(bfp, wsf[j][:, ci * P:(ci + 1) * P], bi_t[j],
                           start=(j == 0), stop=(j == NCH - 1))
                    nc.vector.tensor_add(out=bff_t[ci], in0=bfp, in1=bf_t[ci])

            # ---- x^T tiles (PE transpose), for the Z contraction ----
            ht = []
            for k in range(NK):
                ks = slice(k * P, (k + 1) * P)
                ps = mmps.tile([P, 512], MDT, tag="mm", name="pst")
                for ci in range(NCH):
                    nc.tensor.transpose(
                        out=ps[:, ci * P:(ci + 1) * P], in_=xm[ci][:, ks],
                        identity=idt
                    )
                t = big.tile([P, C], MDT, tag=f"ht{k}", name=f"ht{k}")
                if k % 2 == 0:
                    nc.vector.tensor_copy(out=t, in_=ps)
                else:
                    nc.scalar.copy(out=t, in_=ps)
                ht.append(t)

            # ---- fused q/k projection: QK2 = sc*((WM*sc)@x_q + bM + WM@bi) ----
            qk = [big.tile([P, NQ], MDT, tag=f"qk{i}", name=f"qk{i}")
                  for i in range(NCH)]
            for ci in range(NCH):
                cs = slice(ci * P, (ci + 1) * P)
                for qc in range(NQC):
                    qs = slice(qc * 512, (qc + 1) * 512)
                    ps = mmps.tile([P, 512], F32, tag="mm", name="psqk")
                    for cj in range(NCH):
                        mm(ps, wmt2[cj][:, cs], xm[cj][:, qs],
                           start=(cj == 0), stop=(cj == NCH - 1))
                    if (ci + qc) % 2 == 0:
                        nc.vector.tensor_scalar(
                            out=qk[ci][:, qs], in0=ps, scalar1=sc_t[ci],
                            scalar2=b2_t[ci], op0=mybir.AluOpType.mult,
                            op1=mybir.AluOpType.add,
                        )
                    else:
                        nc.scalar.activation(
                            out=qk[ci][:, qs], in_=ps, func=Id,
                            bias=b2_t[ci], scale=sc_t[ci],
                        )

            wmq_pool.__exit__(None, None, None)

            attn_pools = (
                tc.tile_pool(name="atp", bufs=4),
                tc.tile_pool(name="znp", bufs=1),
                tc.tile_pool(name="nrm", bufs=1),
                tc.tile_pool(name="misc", bufs=2),
            )
            atp = attn_pools[0].__enter__()
            znp = attn_pools[1].__enter__()
            nrm = attn_pools[2].__enter__()
            misc = attn_pools[3].__enter__()

            # ---- attention, one q-chunk of 512 at a time ----
            # The projection+finalize of chunk q is deferred into the middle
            # of chunk q+1's k-loop so the PE never idles on the DVE chain.
            def make_finalize(zn, r, qs):
                def finalize():
                    rbp = mmps.tile([P, 512], F32, tag="mm", name="rbp")
                    mm(rbp, ones_m, r, start=True, stop=True)
                    rb = nrm.tile([P, 512], F32, tag="rb", name="rb")
                    nc.vector.tensor_copy(out=rb, in_=rbp)
                    for co in range(NCH):
                        cs = slice(co * P, (co + 1) * P)
                        xr = misc.tile([P, 512], F32, tag="xr", name="xr")
                        (nc.sync if co % 2 else nc.gpsimd).dma_start(
                            out=xr, in_=xf[cs, qs])
                        fin = mmps.tile([P, 512], F32, tag="mm", name=f"fin{co}")
                        for ci in range(NCH):
                            mm(fin, wft2[ci][:, cs], zn[ci],
                               start=(ci == 0), stop=(ci == NCH - 1))
                        osb = misc.tile([P, 512], F32, tag="osb", name="osb")
                        nc.vector.tensor_mul(out=osb, in0=fin, in1=rb)
                        nc.vector.scalar_tensor_tensor(
                            out=osb, in0=osb, scalar=bff_t[co], in1=xr,
                            op0=Add, op1=Add,
                        )
                        (nc.gpsimd if co % 2 else nc.sync
                         ).dma_start(out=out_d[cs, qs], in_=osb)
                return finalize

            def make_finalize_last(zac, r, qs):
                """Tail-optimized: residuals prefetched, normalizer folded
                into the accumulator copy (it commutes through the
                projection), output step a single DVE op off PSUM."""
                def finalize():
                    xrs = []
                    for co in range(NCH):
                        if co < 2:
                            xr = misc.tile([P, 512], F32, tag=f"xrl{co}",
                                          name=f"xrl{co}", bufs=1)
                        else:
                            xr = misc.tile([P, 512], F32, tag="xr",
                                           name=f"xrl{co}")
                        nc.sync.dma_start(
                            out=xr, in_=xf[co * P:(co + 1) * P, qs])
                        xrs.append(xr)
                    rbp = sps.tile([P, 512], F32, tag="sums", name="rbp")
                    mm(rbp, ones_m, r, start=True, stop=True)
                    rb = nrm.tile([P, 512], F32, tag="rb", name="rb")
                    nc.vector.tensor_copy(out=rb, in_=rbp)
                    zn = []
                    for ci in range(NCH):
                        t = znp.tile([P, 512], MDT, tag=f"zn{ci}",
                                     name=f"znrb{ci}")
                        nc.vector.tensor_mul(out=t, in0=zac[ci], in1=rb)
                        zn.append(t)
                    for co in range(NCH):
                        cs = slice(co * P, (co + 1) * P)
                        fin = mmps.tile([P, 512], F32, tag="mm", name=f"finl{co}")
                        for ci in range(NCH):
                            mm(fin, wft2[ci][:, cs], zn[ci],
                               start=(ci == 0), stop=(ci == NCH - 1))
                        osb = misc.tile([P, 512], F32, tag="osb", name="osb")
                        nc.vector.scalar_tensor_tensor(
                            out=osb, in0=fin, scalar=bff_t[co], in1=xrs[co],
                            op0=Add, op1=Add,
                        )
                        nc.sync.dma_start(out=out_d[cs, qs], in_=osb)
                return finalize

            pending = None
            for qc in range(NQC):
                qs = slice(qc * 512, (qc + 1) * 512)

                def emit_st(k):
                    """S^T[k*P:(k+1)*P, qs] -> exp -> SBUF tile (MDT)."""
                    ks = slice(k * P, (k + 1) * P)
                    st = mmps.tile([P, 512], F32, tag="mm", name="st")
                    for ci in range(NCH):
                        mm(st, xm[ci][:, ks], qk[ci][:, qs],
                           start=(ci == 0), stop=(ci == NCH - 1))
                    at = atp.tile([P, 512], MDT, tag="at", name="at")
                    nc.scalar.activation(out=at, in_=st, func=Exp,
                                         scale=SM_SCALE)
                    return at

                zac = [zps.tile([P, 512], F32, tag=f"z{ci}", name=f"zac{ci}")
                       for ci in range(NCH)]
                sums = sps.tile([1, 512], F32, tag="sums", name="sums")

                at_cur = emit_st(0)
                for k in range(NK):
                    if k == 3 and pending is not None:
                        pending()
                        pending = None
                    at_next = emit_st(k + 1) if k + 1 < NK else None
                    first, last = (k == 0), (k == NK - 1)
                    for ci in range(NCH):
                        mm(zac[ci], ht[k][:, ci * P:(ci + 1) * P], at_cur,
                           start=first, stop=last)
                    mm(sums, ones_k, at_cur, start=first, stop=last)
                    at_cur = at_next

                # epilogue A: compute r; free the accumulators (non-last)
                r = nrm.tile([1, 512], MDT, tag="r", name="r")
                with nc.allow_low_precision(reason="fp32r normalizer"):
                    nc.vector.reciprocal(out=r, in_=sums)
                if qc == NQC - 1:
                    pending = make_finalize_last(zac, r, qs)
                else:
                    zn = []
                    for ci in range(NCH):
                        t = znp.tile([P, 512], MDT, tag=f"zn{ci}",
                                     name=f"zn{ci}")
                        nc.vector.tensor_copy(out=t, in_=zac[ci])
                        zn.append(t)
                    pending = make_finalize(zn, r, qs)
            pending()

            for pcm in reversed(attn_pools):
                pcm.__exit__(None, None, None)

    nc.compile()
    _CACHE["nc"] = nc
    return nc


def make_in_maps(x, gn_gamma, gn_beta, wq, bq, wk, bk, wv, bv, wo, bo):
    """Host preprocessing + per-core input maps. bk drops out exactly
    (softmax shift invariance)."""
    f = np.float32
    x = np.asarray(x, f).reshape(4, C, N)
    wq, wk, wv, wo = (np.asarray(w, f) for w in (wq, wk, wv, wo))
    bq, bv, bo = (np.asarray(b, f) for b in (bq, bv, bo))

    wmt = np.ascontiguousarray(wq.T @ wk)          # [cj, ci]
    wft = np.ascontiguousarray((wo @ wv).T)        # [ci, co]
    biasc = np.stack(
        [wk.T @ bq, wo @ bv + bo,
         np.asarray(gn_gamma, f), np.asarray(gn_beta, f)], axis=1
    ).astype(f)                                    # [C, 4]: bm, bf, gamma, beta

    g = np.zeros((P, P), f)
    for i in range(0, P, GROUP):
        g[i:i + GROUP, i:i + GROUP] = 1.0
    idt = np.eye(P, dtype=f)

    shared = dict(wmt=wmt, wft=wft, biasc=biasc, gmat=g, idt=idt)
    in_maps = []
    for core in range(8):
        b, half = core // 2, core % 2
        xs = x[b]
        if half:
            xs = np.ascontiguousarray(
                np.concatenate([xs[:, NQ:], xs[:, :NQ]], axis=1)
            )
        in_maps.append(dict(shared, xf=xs))
    return in_maps


def assemble(results):
    out = np.empty((4, C, N), np.float32)
    for core in range(8):
        b, half = core // 2, core % 2
        out[b, :, half * NQ:(half + 1) * NQ] = results[core]["out"]
    return out.reshape(4, C, 64, 64)


def kernel(**inputs):
    nc = build_module()
    in_maps = make_in_maps(**inputs)
    res = run_bass_kernel_spmd(nc, in_maps, list(range(8)))
    return assemble(res.results)
